# revision 2
# baseline (speedup 1.0000x reference)
"""Trainium2 Bass kernel for CombinedCriterion (chamfer + Sinkhorn-EMD loss).

Strategy (pure data parallel, 8 cores x 4 batch items):
  - The log-domain Sinkhorn of the reference is algebraically identical to
    standard Sinkhorn on the Gibbs kernel Mt = N*exp(-C/eps):
        u = 1/(Mt v),  v = 1/(Mt^T u)
    so each iteration is two PE matrix-vector products + tiny reciprocals.
  - Mt is built once per item: T = 2x.y^T - yy (K=4 augmented fp32 matmul),
    then ACT exp with per-partition bias (-xx/eps + ln N), stored bf16 in both
    layouts (i-major and j-major) for the two matvec directions.
  - Chamfer mins fall out of VE reduce_max over the T psum tiles (fp32 exact).
  - The final transport cost decomposes via C = xx + yy - 2x.y into 4 extra
    matvecs (q = Mt @ [v, v*y0, v*y1, v*y2]) evaluated as one N=8 matmul pass
    with hi/lo bf16-split right-hand sides; the scalar reduction happens on
    the host in fp64 from tiny [128,8] tensors.
"""

import numpy as np
from contextlib import ExitStack

import concourse.bass as bass
import concourse.tile as tile
from concourse import bacc, mybir
from concourse import bass_utils

F32 = mybir.dt.float32
BF16 = mybir.dt.bfloat16

B, N, D = 32, 1024, 3
NCORES = 8
IPC = B // NCORES          # items per core
NT = N // 128              # 8 partition tiles
EPS = 0.1
ITERS = 20
BETA = 0.01
Z_SCALES = np.array([2.0, 4.0, 8.0, 16.0, 32.0], dtype=np.float64)
Z_DIM = 16.0
LNN = float(np.log(float(N)))
REPEAT = 1                 # >1: wrap body in a device-side For_i (bench only)

_CACHE = {}


def _emit_kernel(tc, ctx, ins, outs, pools):
    nc = tc.nc
    X = mybir.AxisListType.X
    consts, mats, vecs, bpsum, mvpsum = pools

    lhs_sb = [[None, None] for _ in range(IPC)]   # [b][side]
    rhs_sb = [[None, None] for _ in range(IPC)]
    bias_sb = [[None, None] for _ in range(IPC)]
    tmax_sb = [[None, None] for _ in range(IPC)]
    ypm_sb, M_sb, MT_sb = [], [], []
    u_f32, v_f32, u_bf, v_bf, q_sb = [], [], [], [], []

    for b in range(IPC):
        for side, (ln, rn, bn) in enumerate(
                [("lhsx", "rhsy", "biasx"), ("lhsy", "rhsx", "biasy")]):
            t = consts.tile([4, N], F32, name=f"{ln}{b}")
            nc.sync.dma_start(t, ins[ln][b])
            lhs_sb[b][side] = t
            t = consts.tile([4, N], F32, name=f"{rn}{b}")
            nc.sync.dma_start(t, ins[rn][b])
            rhs_sb[b][side] = t
            t = consts.tile([128, NT], F32, name=f"{bn}{b}")
            nc.sync.dma_start(t, ins[bn][b])
            bias_sb[b][side] = t
            tmax_sb[b][side] = vecs.tile([128, NT], F32, name=f"tm{side}_{b}")
        t = consts.tile([128, NT, D], F32, name=f"ypm{b}")
        nc.sync.dma_start(t, ins["ypm"][b])
        ypm_sb.append(t)

        M_sb.append(mats.tile([128, NT * N], BF16, name=f"M{b}"))
        MT_sb.append(mats.tile([128, NT * N], BF16, name=f"MT{b}"))
        u_f32.append(vecs.tile([128, NT], F32, name=f"uf{b}"))
        v_f32.append(vecs.tile([128, NT], F32, name=f"vf{b}"))
        u_bf.append(vecs.tile([128, NT], BF16, name=f"ub{b}"))
        v_bf.append(vecs.tile([128, NT], BF16, name=f"vb{b}"))
        q_sb.append(vecs.tile([128, NT * 8], F32, name=f"q{b}"))

        nc.vector.memset(v_bf[b], 1.0)

    # ---- build Mt: MT (side 1, needed first by u-dir) then M (side 0) ----
    for side in (1, 0):
        for b in range(IPC):
            dst = MT_sb[b] if side == 1 else M_sb[b]
            lhs, rhs, bia, tmx = (lhs_sb[b][side], rhs_sb[b][side],
                                  bias_sb[b][side], tmax_sb[b][side])
            for it in range(NT):
                ps = bpsum.tile([128, N], F32, tag="bld", name=f"bld{b}_{side}_{it}")
                for h in range(2):
                    nc.tensor.matmul(
                        ps[:, h * 512:(h + 1) * 512],
                        lhs[:, it * 128:(it + 1) * 128],
                        rhs[:, h * 512:(h + 1) * 512],
                        start=True, stop=True)
                nc.scalar.activation(
                    out=dst[:, it * N:(it + 1) * N], in_=ps,
                    func=mybir.ActivationFunctionType.Exp,
                    bias=bia[:, it:it + 1], scale=1.0 / EPS)
                nc.vector.reduce_max(out=tmx[:, it:it + 1], in_=ps, axis=X)

    # ---- Sinkhorn iterations (round-robin across items) ----
    def final_q(b):
        R = vecs.tile([128, NT, 8], BF16, tag="R", bufs=2, name=f"R{b}")
        vh32 = vecs.tile([128, NT], F32, tag="vh32", bufs=2, name=f"vh32_{b}")
        nc.scalar.copy(out=R[:, :, 0], in_=v_f32[b])
        nc.scalar.copy(out=vh32, in_=R[:, :, 0])
        nc.vector.tensor_sub(R[:, :, 1], v_f32[b], vh32)
        for d in range(D):
            wd = vecs.tile([128, NT], F32, tag="wd", bufs=2, name=f"wd{b}_{d}")
            nc.vector.tensor_mul(wd, v_f32[b], ypm_sb[b][:, :, d])
            nc.scalar.copy(out=R[:, :, 2 + 2 * d], in_=wd)
            nc.scalar.copy(out=vh32, in_=R[:, :, 2 + 2 * d])
            nc.vector.tensor_sub(R[:, :, 3 + 2 * d], wd, vh32)
        for it in range(NT):
            psq = mvpsum.tile([128, 8], F32, tag="mv", name=f"psq{b}_{it}")
            for jt in range(NT):
                nc.tensor.matmul(
                    psq,
                    MT_sb[b][:, jt * N + it * 128: jt * N + (it + 1) * 128],
                    R[:, jt, :],
                    start=(jt == 0), stop=(jt == NT - 1))
            nc.vector.tensor_copy(q_sb[b][:, it * 8:(it + 1) * 8], psq)
        nc.sync.dma_start(outs["qout"][b], q_sb[b])
        nc.sync.dma_start(outs["ufin"][b], u_f32[b])
        nc.sync.dma_start(outs["tmaxx"][b], tmax_sb[b][0])
        nc.sync.dma_start(outs["tmaxy"][b], tmax_sb[b][1])

    with nc.allow_low_precision("sinkhorn scaling vectors are bf16 by design"):
        for t in range(ITERS):
            last = t == ITERS - 1
            for b in range(IPC):
                psu = mvpsum.tile([128, NT], F32, tag="mv", name=f"psu{t}_{b}")
                for it in range(NT):
                    for jt in range(NT):
                        nc.tensor.matmul(
                            psu[:, it:it + 1],
                            MT_sb[b][:, jt * N + it * 128: jt * N + (it + 1) * 128],
                            v_bf[b][:, jt:jt + 1],
                            start=(jt == 0), stop=(jt == NT - 1))
                if last:
                    nc.vector.reciprocal(out=u_f32[b], in_=psu)
                    nc.scalar.copy(out=u_bf[b], in_=u_f32[b])
                else:
                    nc.vector.reciprocal(out=u_bf[b], in_=psu)
            for b in range(IPC):
                psv = mvpsum.tile([128, NT], F32, tag="mv", name=f"psv{t}_{b}")
                for jt in range(NT):
                    for it in range(NT):
                        nc.tensor.matmul(
                            psv[:, jt:jt + 1],
                            M_sb[b][:, it * N + jt * 128: it * N + (jt + 1) * 128],
                            u_bf[b][:, it:it + 1],
                            start=(it == 0), stop=(it == NT - 1))
                if last:
                    nc.vector.reciprocal(out=v_f32[b], in_=psv)
                    final_q(b)
                else:
                    nc.vector.reciprocal(out=v_bf[b], in_=psv)


def _get_module():
    key = ("nc", ITERS, REPEAT)
    if key in _CACHE:
        return _CACHE[key]
    nc = bacc.Bacc("TRN2", target_bir_lowering=False, debug=False,
                   enable_asserts=False, num_devices=NCORES)
    ins = {}
    for name, shape in [("lhsx", [IPC, 4, N]), ("rhsy", [IPC, 4, N]),
                        ("lhsy", [IPC, 4, N]), ("rhsx", [IPC, 4, N]),
                        ("biasx", [IPC, 128, NT]), ("biasy", [IPC, 128, NT]),
                        ("ypm", [IPC, 128, NT, D])]:
        ins[name] = nc.dram_tensor(name, shape, F32, kind="ExternalInput").ap()
    outs = {}
    for name, shape in [("tmaxx", [IPC, 128, NT]), ("tmaxy", [IPC, 128, NT]),
                        ("ufin", [IPC, 128, NT]), ("qout", [IPC, 128, NT * 8])]:
        outs[name] = nc.dram_tensor(name, shape, F32, kind="ExternalOutput").ap()
    with tile.TileContext(nc) as tc, ExitStack() as ctx:
        pools = (
            ctx.enter_context(tc.tile_pool(name="consts", bufs=1)),
            ctx.enter_context(tc.tile_pool(name="mats", bufs=1)),
            ctx.enter_context(tc.tile_pool(name="vecs", bufs=1)),
            ctx.enter_context(tc.tile_pool(name="bpsum", bufs=2, space="PSUM")),
            ctx.enter_context(tc.tile_pool(name="mvpsum", bufs=4, space="PSUM")),
        )
        if REPEAT > 1:
            with tc.For_i(0, REPEAT, 1):
                _emit_kernel(tc, ctx, ins, outs, pools)
        else:
            _emit_kernel(tc, ctx, ins, outs, pools)
    nc.compile()
    _CACHE[key] = nc
    return nc


def _pm(a):
    """[N] -> [128, NT] partition-major (i = t*128 + p)."""
    return np.ascontiguousarray(a.reshape(NT, 128).T)


def _prep_core(xs, ys):
    """xs, ys: [IPC, N, D] fp32 -> input dict for one core."""
    m = {k: [] for k in ("lhsx", "rhsy", "lhsy", "rhsx", "biasx", "biasy", "ypm")}
    ones = np.ones((1, N), np.float32)
    for b in range(IPC):
        x, y = xs[b], ys[b]
        xx = np.sum(x * x, -1)
        yy = np.sum(y * y, -1)
        m["lhsx"].append(np.concatenate([2.0 * x.T, ones], 0))
        m["rhsy"].append(np.concatenate([y.T, -yy[None]], 0))
        m["lhsy"].append(np.concatenate([2.0 * y.T, ones], 0))
        m["rhsx"].append(np.concatenate([x.T, -xx[None]], 0))
        m["biasx"].append(_pm(-xx / EPS + LNN))
        m["biasy"].append(_pm(-yy / EPS + LNN))
        m["ypm"].append(np.ascontiguousarray(y.reshape(NT, 128, D).transpose(1, 0, 2)))
    return {k: np.stack(v).astype(np.float32) for k, v in m.items()}


def _run_device(inputs, trace=False):
    nc = _get_module()
    xs = np.asarray(inputs["output_set"], np.float32)
    ys = np.asarray(inputs["target_set"], np.float32)
    in_maps = [_prep_core(xs[c * IPC:(c + 1) * IPC], ys[c * IPC:(c + 1) * IPC])
               for c in range(NCORES)]
    res = bass_utils.run_bass_kernel_spmd(
        nc, in_maps, core_ids=list(range(NCORES)), trace=trace)
    return res


def _combine(inputs, results):
    xs = np.asarray(inputs["output_set"], np.float64)
    ys = np.asarray(inputs["target_set"], np.float64)
    kls = np.asarray(inputs["kls"], np.float64)
    cds, emds = [], []
    for c in range(NCORES):
        r = results[c]
        for b in range(IPC):
            gb = c * IPC + b
            x, y = xs[gb], ys[gb]
            xx = np.sum(x * x, -1)
            yy = np.sum(y * y, -1)
            tmx = r["tmaxx"][b].T.reshape(N).astype(np.float64)
            tmy = r["tmaxy"][b].T.reshape(N).astype(np.float64)
            cds.append((xx - tmx).mean() + (yy - tmy).mean())
            u = r["ufin"][b].T.reshape(N).astype(np.float64)
            q8 = r["qout"][b].reshape(128, NT, 8).transpose(1, 0, 2).reshape(N, 8)
            q8 = q8.astype(np.float64)
            q0 = q8[:, 0] + q8[:, 1]
            emd = np.sum(u * xx * q0) + np.sum(yy)
            for d in range(D):
                qd = q8[:, 2 + 2 * d] + q8[:, 3 + 2 * d]
                emd -= 2.0 * np.sum(u * x[:, d] * qd)
            emds.append(emd / N)
    cd = float(np.mean(cds))
    emd = float(np.mean(emds))
    recon = cd + emd
    kl = float(np.mean(np.sum(kls, 1)))
    loss = BETA * kl + recon
    td = (kls.mean(0) / (Z_SCALES * Z_DIM)).astype(np.float32)
    return (np.float32(loss), np.float32(kl), np.float32(recon), td)


def kernel(output_set, output_mask, target_set, target_mask, kls):
    inputs = {"output_set": output_set, "output_mask": output_mask,
              "target_set": target_set, "target_mask": target_mask, "kls": kls}
    res = _run_device(inputs, trace=False)
    return _combine(inputs, res.results)
